# revision 1
# baseline (speedup 1.0000x reference)
"""Trainium2 Bass kernel for CAN multi-head message passing (GAT-style).

Strategy (vertex-cut by TARGET node, 8 cores):
  - Edges are sorted by target and sharded so core c owns target nodes
    [c*6250, (c+1)*6250). Each core fully computes its own output rows;
    no cross-core reduction is needed.
  - Phase A (per core, replicated): x_msg = x @ W for all 4 heads, plus the
    per-node attention scalars s_n = x_msg[n,h,:]@aw_s[h], t_n = ...@aw_t[h],
    written to DRAM as fp16 rows [msg(256) | s(4) | t(4) | pad] (768B, wrapped
    layout); the t-gather reads the 256B-aligned column slice [256:384] of the
    same table with elem_step=384, so no separate t table is needed.
  - Phase B: per 128-target-node window, per-edge rows are fetched with the
    GPSIMD dma_gather extended instruction.  int16 index limitation is beaten
    by storing node n at physical row (n+32768) % 65536 and pointing the
    gather base at row 32768: signed int16 wraparound then addresses all 50k
    nodes (verified on HW).  Gathers are capped at 1024 indices (descriptor
    ring limit).
  - softmax (no max-subtraction needed: |z| <= ~10 for this data; a constant
    bias of -4 inside Exp guards fp16 range; constants cancel in softmax) and
    aggregation via one-hot matmuls: for each chunk of 128 edges, a [128e x
    128n] one-hot of local targets is built with a DVE is_equal and a single
    PE matmul accumulates both the weighted messages (256 cols) and the
    denominators (4 cols) into PSUM across all chunks of the window.
"""
import sys
sys.path.insert(0, "/opt/trn_rl_repo")
import numpy as np

N_NODES = 50000
N_EDGES = 1600000
IN_CH = 128
OUT_CH = 64
N_HEADS = 4
HO = N_HEADS * OUT_CH          # 256
NCORES = 8
NPC = N_NODES // NCORES        # 6250 nodes per core
NW = 49                        # windows per core (48*128 + 106)
XROW = 384                     # fp16 elems per gathered row (768B): msg|s|t|pad
TROW = 128                     # fp16 elems per t-gather slice (256B)
SEG = 1024                     # max indices per dma_gather
SEGC = SEG // 128              # 8 chunks per segment
EXP_BIAS = -4.0


def _pack_idx(flat_i16: np.ndarray) -> np.ndarray:
    """[1024] int16 -> [128, 64] idx tile (idx j at [j%16, j//16], x8 replicas)."""
    a = flat_i16.reshape(SEG // 16, 16).T
    return np.tile(a, (8, 1))


def _host_prep(x_source, edge_tgt, edge_src, edge_vals, weight, att_weight):
    perm = np.argsort(edge_tgt, kind="stable")
    tgt_s = np.asarray(edge_tgt)[perm].astype(np.int64)
    src_s = np.asarray(edge_src)[perm].astype(np.int64)
    val_s = np.asarray(edge_vals)[perm].astype(np.float32)

    core_edge_bounds = np.searchsorted(tgt_s, np.arange(NCORES + 1) * NPC)

    # window edge counts -> Cmax
    win_starts = []   # per (core, w): slice into sorted arrays
    max_cnt = 0
    for c in range(NCORES):
        for w in range(NW):
            n0 = c * NPC + w * 128
            n1 = min(c * NPC + (w + 1) * 128, (c + 1) * NPC)
            a = np.searchsorted(tgt_s, n0)
            b = np.searchsorted(tgt_s, n1)
            win_starts.append((c, w, n0, a, b))
            max_cnt = max(max_cnt, b - a)
    # reserved last-slot-per-segment costs ~Cmax/8 slots per window
    Cmax = (max_cnt + 8 + 127) // 128
    while Cmax * 128 - ((Cmax + SEGC - 1) // SEGC + 1) < max_cnt:
        Cmax += 1
    TC = NW * Cmax                      # chunks per core
    TSEG = (TC + SEGC - 1) // SEGC      # gather segments per core

    src_i16 = np.zeros((NCORES, TC, 128), np.int16)
    tgt_i16 = np.zeros((NCORES, TC, 128), np.int16)
    tgtl = np.full((NCORES, NW, 128, Cmax), 200.0, np.float16)
    vals = np.zeros((NCORES, NW, 128, Cmax), np.float32)

    for (c, w, n0, a, b) in win_starts:
        cnt = b - a
        if cnt == 0:
            continue
        gc0 = w * Cmax
        # slot j = c_rel*128 + p, skipping reserved slots (global chunk
        # gc0+c_rel with (gc0+c_rel) % SEGC == SEGC-1 and p == 127)
        slots = np.arange(Cmax * 128)
        gcs = gc0 + slots // 128
        resv = ((gcs % SEGC) == SEGC - 1) & ((slots % 128) == 127)
        slots = slots[~resv][:cnt]
        assert len(slots) == cnt, (c, w, cnt, Cmax)
        crel = slots // 128
        p = slots % 128
        src_i16[c, gc0 + crel, p] = src_s[a:b].astype(np.int16)
        tgt_i16[c, gc0 + crel, p] = tgt_s[a:b].astype(np.int16)
        tgtl[c, w, p, crel] = (tgt_s[a:b] - n0).astype(np.float16)
        vals[c, w, p, crel] = val_s[a:b]

    # segment-packed idx arrays
    idx_src = np.zeros((NCORES, TSEG, 128, SEG // 16), np.int16)
    idx_tgt = np.zeros((NCORES, TSEG, 128, SEG // 16), np.int16)
    for c in range(NCORES):
        flat_s = np.zeros(TSEG * SEG, np.int16)
        flat_t = np.zeros(TSEG * SEG, np.int16)
        flat_s[:TC * 128] = src_i16[c].reshape(-1)
        flat_t[:TC * 128] = tgt_i16[c].reshape(-1)
        for s in range(TSEG):
            idx_src[c, s] = _pack_idx(flat_s[s * SEG:(s + 1) * SEG])
            idx_tgt[c, s] = _pack_idx(flat_t[s * SEG:(s + 1) * SEG])

    # weights: wcat [128, 264] = [W (i->(h,o)) | ws | wt]
    W = np.asarray(weight, np.float32)              # [4, 128, 64]
    aw = np.asarray(att_weight, np.float32)         # [4, 128]
    ws = np.stack([W[h] @ aw[h, :OUT_CH] for h in range(N_HEADS)], 1)   # [128,4]
    wt = np.stack([W[h] @ aw[h, OUT_CH:] for h in range(N_HEADS)], 1)
    wcat = np.concatenate([W.transpose(1, 0, 2).reshape(IN_CH, HO), ws, wt], 1)

    x_T = np.ascontiguousarray(np.asarray(x_source, np.float32).T)  # [128, 50000]

    tgtl = np.ascontiguousarray(tgtl.transpose(0, 2, 1, 3))  # [C,128,NW,Cmax]
    vals = np.ascontiguousarray(vals.transpose(0, 2, 1, 3))
    return dict(Cmax=Cmax, TC=TC, TSEG=TSEG, x_T=x_T, wcat=wcat,
                idx_src=idx_src, idx_tgt=idx_tgt, tgtl=tgtl, vals=vals)


def _build(Cmax, TC, TSEG):
    KNW = NW
    import concourse.bass as bass
    import concourse.tile as tile
    from concourse import bacc, mybir

    f32, f16, i16, i32 = (mybir.dt.float32, mybir.dt.float16,
                          mybir.dt.int16, mybir.dt.int32)
    Alu = mybir.AluOpType
    Act = mybir.ActivationFunctionType

    nc = bacc.Bacc("TRN2", target_bir_lowering=False, debug=False,
                   num_devices=NCORES, num_swdge_queues=1)
    x_T = nc.dram_tensor("x_T", [IN_CH, N_NODES], f32, kind="ExternalInput")
    wcat = nc.dram_tensor("wcat", [IN_CH, HO + 8], f32, kind="ExternalInput")
    idx_src = nc.dram_tensor("idx_src", [TSEG, 128, SEG // 16], i16,
                             kind="ExternalInput")
    idx_tgt = nc.dram_tensor("idx_tgt", [TSEG, 128, SEG // 16], i16,
                             kind="ExternalInput")
    tgtl_in = nc.dram_tensor("tgtl", [128, NW, Cmax], f16, kind="ExternalInput")
    vals_in = nc.dram_tensor("vals", [128, NW, Cmax], f32, kind="ExternalInput")
    out_d = nc.dram_tensor("out", [NPC, HO], f32, kind="ExternalOutput")
    xw = nc.dram_tensor("xw", [65536, XROW], f16, kind="Internal")

    NT = (N_NODES + 127) // 128   # 391 node tiles

    with tile.TileContext(nc) as tc:
        # ---------------- phase A ----------------
        with tc.tile_pool(name="a_w", bufs=1) as cpool, \
             tc.tile_pool(name="a_x", bufs=4) as xpool, \
             tc.tile_pool(name="a_ps", bufs=4, space="PSUM") as apsum, \
             tc.tile_pool(name="a_m", bufs=4) as mpool:
            wc = cpool.tile([128, HO + 8], f32)
            nc.sync.dma_start(wc[:], wcat[:])
            for i in range(NT):
                rows = min(128, N_NODES - i * 128)
                xt = xpool.tile([128, 128], f32)
                nc.sync.dma_start(xt[:, 0:rows], x_T[:, i * 128:i * 128 + rows])
                ps = apsum.tile([128, HO + 8], f32)
                nc.tensor.matmul(ps[0:rows, :], xt[:, 0:rows], wc[:])
                m = mpool.tile([128, HO + 8], f16, tag="m")
                nc.vector.tensor_copy(m[0:rows, :], ps[0:rows, 0:HO + 8])
                r0 = (i * 128 + 32768) % 65536
                nc.sync.dma_start(xw[r0:r0 + rows, 0:HO + 8], m[0:rows, :])

        # ---------------- phase B ----------------
        with tc.tile_pool(name="b_c", bufs=1) as bconst, \
             tc.tile_pool(name="b_idx", bufs=16) as idxp, \
             tc.tile_pool(name="b_g", bufs=16) as gpool, \
             tc.tile_pool(name="b_t", bufs=16) as tpool, \
             tc.tile_pool(name="b_l", bufs=3) as lpool, \
             tc.tile_pool(name="b_z", bufs=3) as zpool, \
             tc.tile_pool(name="b_oh", bufs=6) as ohpool, \
             tc.tile_pool(name="b_ps", bufs=2, space="PSUM") as bpsum, \
             tc.tile_pool(name="b_o", bufs=4) as opool:

            it32 = bconst.tile([128, 4 * 128], i32)
            nc.gpsimd.iota(it32[:], pattern=[[0, 4], [1, 128]],
                           channel_multiplier=0)
            iota4 = bconst.tile([128, 4, 128], f16)
            nc.vector.tensor_copy(iota4[:].rearrange("p a b -> p (a b)"), it32[:])
            bias_t = bconst.tile([128, 1], f32)
            nc.vector.memset(bias_t[:], EXP_BIAS)
            tl_all = bconst.tile([128, NW, Cmax], f16)
            nc.sync.dma_start(tl_all[:], tgtl_in[:])
            vv_all = bconst.tile([128, NW, Cmax], f32)
            nc.sync.dma_start(vv_all[:], vals_in[:])

            tc.strict_bb_all_engine_barrier()

            seg_tiles = {}

            def get_seg(s):
                if s not in seg_tiles:
                    si = idxp.tile([128, SEG // 16], i16, tag="si")
                    nc.sync.dma_start(si[:], idx_src[s])
                    ti = idxp.tile([128, SEG // 16], i16, tag="ti")
                    nc.sync.dma_start(ti[:], idx_tgt[s])
                    g = gpool.tile([128, SEGC, XROW], f16)
                    nc.gpsimd.dma_gather(g[:], xw[32768:, :], si[:], SEG, SEG,
                                         XROW, queue_num=0)
                    tg = tpool.tile([128, SEGC, TROW], f16)
                    nc.gpsimd.dma_gather(tg[:], xw[32768:, HO:HO + TROW], ti[:],
                                         SEG, SEG, TROW, elem_step=XROW,
                                         queue_num=0)
                    seg_tiles[s] = (g, tg)
                return seg_tiles[s]

            def bc(apv, n):
                return bass.AP(apv.tensor, apv.offset, list(apv.ap) + [[0, n]])

            for w in range(KNW):
                rows = min(128, NPC - w * 128)
                tl = tl_all[:, w, :]
                vv = vv_all[:, w, :]

                gc0, gc1 = w * Cmax, (w + 1) * Cmax
                segs = sorted({gc // SEGC for gc in range(gc0, gc1)})

                # z = s + t (per segment range)
                z = zpool.tile([128, Cmax, N_HEADS], f32, tag="z")
                for s in segs:
                    lo, hi = max(s * SEGC, gc0), min(s * SEGC + SEGC, gc1)
                    g, tg = get_seg(s)
                    nc.vector.tensor_tensor(
                        z[:, lo - gc0:hi - gc0, :],
                        g[:, lo - s * SEGC:hi - s * SEGC, HO:HO + 4],
                        tg[:, lo - s * SEGC:hi - s * SEGC, 4:8], op=Alu.add)
                # lrelu
                zz = zpool.tile([128, Cmax, N_HEADS], f32, tag="zz")
                nc.vector.scalar_tensor_tensor(
                    zz[:].rearrange("p c h -> p (c h)"),
                    z[:].rearrange("p c h -> p (c h)"), 0.01,
                    z[:].rearrange("p c h -> p (c h)"),
                    op0=Alu.mult, op1=Alu.max)
                # * vals
                nc.vector.tensor_tensor(zz[:], zz[:], bc(vv, N_HEADS),
                                        op=Alu.mult)
                # p = exp(zz - 4)
                p = zpool.tile([128, Cmax, N_HEADS], f16, tag="p")
                nc.scalar.activation(p[:], zz[:], Act.Exp, bias=bias_t[:])

                # rhs in-place: g.msg *= p ; g.s <- p
                for s in segs:
                    lo, hi = max(s * SEGC, gc0), min(s * SEGC + SEGC, gc1)
                    g, _ = get_seg(s)
                    n = hi - lo
                    gm = g[:, lo - s * SEGC:hi - s * SEGC, 0:HO].rearrange(
                        "p c (h o) -> p c h o", o=OUT_CH)
                    nc.vector.tensor_tensor(
                        gm, gm, bc(p[:, lo - gc0:hi - gc0, :], OUT_CH),
                        op=Alu.mult)
                    nc.vector.tensor_copy(
                        g[:, lo - s * SEGC:hi - s * SEGC, HO:HO + 4],
                        p[:, lo - gc0:hi - gc0, :])

                ps = bpsum.tile([128, HO + 4], f32)
                for cb in range(0, Cmax, 4):
                    nb = min(4, Cmax - cb)
                    oh = ohpool.tile([128, 4, 128], f16)
                    nc.vector.tensor_tensor(
                        oh[:, 0:nb, :], iota4[:, 0:nb, :],
                        bc(tl[:, cb:cb + nb], 128), op=Alu.is_equal)
                    for j in range(nb):
                        c = cb + j
                        gc = gc0 + c
                        g, _ = get_seg(gc // SEGC)
                        nc.tensor.matmul(
                            ps[:], oh[:, j, :],
                            g[:, gc % SEGC, 0:HO + 4],
                            start=(c == 0), stop=(c == Cmax - 1))

                d = opool.tile([128, 4], f32, tag="d")
                nc.vector.tensor_scalar_max(d[:], ps[:, HO:HO + 4], 1e-30)
                r = opool.tile([128, 4], f32, tag="r")
                nc.vector.reciprocal(r[:], d[:])
                o = opool.tile([128, HO], f32, tag="o")
                nc.vector.tensor_tensor(
                    o[:].rearrange("p (h q) -> p h q", q=OUT_CH),
                    ps[:, 0:HO].rearrange("p (h q) -> p h q", q=OUT_CH),
                    bc(r[:], OUT_CH), op=Alu.mult)
                nc.sync.dma_start(out_d[w * 128:w * 128 + rows, :], o[0:rows, :])

    nc.finalize()
    return nc


_CACHE = {}


def kernel(x_source, edge_tgt, edge_src, edge_vals, weight, att_weight):
    from concourse import bass_utils

    prep = _host_prep(np.asarray(x_source), np.asarray(edge_tgt),
                      np.asarray(edge_src), np.asarray(edge_vals),
                      np.asarray(weight), np.asarray(att_weight))
    key = (prep["Cmax"], prep["TC"], prep["TSEG"])
    if key not in _CACHE:
        _CACHE[key] = _build(*key)
    nc = _CACHE[key]

    in_maps = []
    for c in range(NCORES):
        in_maps.append({
            "x_T": prep["x_T"], "wcat": prep["wcat"],
            "idx_src": prep["idx_src"][c], "idx_tgt": prep["idx_tgt"][c],
            "tgtl": prep["tgtl"][c], "vals": prep["vals"][c],
        })
    import time
    t0 = time.time()
    res = bass_utils.run_bass_kernel_spmd(nc, in_maps,
                                          core_ids=list(range(NCORES)))
    kernel.last_run_wall_s = time.time() - t0
    out = np.empty((N_NODES, HO), np.float32)
    for c in range(NCORES):
        out[c * NPC:(c + 1) * NPC, :] = res.results[c]["out"]
    return out



# revision 3
# speedup vs baseline: 6.0838x; 6.0838x over previous
"""Trainium2 Bass kernel for CAN multi-head message passing (GAT-style).

Strategy (vertex-cut by TARGET node, 8 cores):
  - Edges are sorted by target and sharded so core c owns target nodes
    [c*6250, (c+1)*6250). Each core fully computes its own output rows.
  - Phase A (sharded): core c receives only ITS slice of x (fp16), computes
    x_msg rows [6250, 264] = [msg(256) | s(4) | t(4)] for its nodes, then an
    8-core DRAM AllGather assembles the full 50000-row table on every core.
    This cuts host->device traffic 8x vs replicating x (the axon tunnel at
    ~50 MB/s is the wall; on-device interconnect is orders faster).
  - Phase B: per 128-target-node window, per-edge rows are fetched with the
    GPSIMD dma_gather extended instruction.  int16 indices are stored as
    (n - 32768) with the gather base at row 32768, addressing all 50k rows
    of the un-scrambled table; the last index of every 1024-index segment is
    a reserved non-negative slot so trailing-negative-index early-exit never
    fires.  Gather index tiles are shipped compact [16, 64] per segment and
    replicated to [128, 64] on device with a 0-stride broadcast DMA (8x
    fewer bytes).
  - softmax (constant bias -4 inside Exp; constants cancel) and aggregation
    via one-hot matmuls accumulate weighted messages (256 cols) and the
    denominators (4 cols) into PSUM across all chunks of a window.
  - edge_vals multiply is skipped entirely when all values are 1.0.
  - Output returned as fp16 (halves device->host traffic), cast to f32 on
    host.  The jitted executable is cached across calls.
"""
import sys
sys.path.insert(0, "/opt/trn_rl_repo")
import numpy as np

N_NODES = 50000
N_EDGES = 1600000
IN_CH = 128
OUT_CH = 64
N_HEADS = 4
HO = N_HEADS * OUT_CH          # 256
NCORES = 8
NPC = N_NODES // NCORES        # 6250 nodes per core
NW = 49                        # windows per core (48*128 + 106)
NPCP = NW * 128                # 6272, padded local node count
XROW = 384                     # fp16 elems per gathered row (768B): msg|s|t|pad
TROW = 128                     # fp16 elems per t-gather slice (256B)
SEG = 1024                     # max indices per dma_gather
SEGC = SEG // 128              # 8 chunks per segment
EXP_BIAS = -4.0


def _host_prep(x_source, edge_tgt, edge_src, edge_vals, weight, att_weight):
    perm = np.argsort(edge_tgt, kind="stable")
    tgt_s = np.asarray(edge_tgt)[perm].astype(np.int64)
    src_s = np.asarray(edge_src)[perm].astype(np.int64)
    novals = bool(np.all(np.asarray(edge_vals) == 1.0))
    val_s = None if novals else np.asarray(edge_vals)[perm].astype(np.float32)

    # window bounds: (core c, window w) covers targets [n0, n1)
    cws = [(c, w) for c in range(NCORES) for w in range(NW)]
    n0s = np.array([c * NPC + w * 128 for c, w in cws])
    n1s = np.minimum(n0s + 128, np.array([(c + 1) * NPC for c, _ in cws]))
    a_s = np.searchsorted(tgt_s, n0s)
    b_s = np.searchsorted(tgt_s, n1s)
    max_cnt = int((b_s - a_s).max())
    Cmax = (max_cnt + 8 + 127) // 128
    while Cmax * 128 - ((Cmax + SEGC - 1) // SEGC + 1) < max_cnt:
        Cmax += 1
    TC = NW * Cmax                      # chunks per core
    TSEG = (TC + SEGC - 1) // SEGC      # gather segments per core

    src_i16 = np.zeros((NCORES, TC, 128), np.int16)
    tgt_i16 = np.zeros((NCORES, TC, 128), np.int16)
    tgtl = np.full((NCORES, NW, 128, Cmax), 200.0, np.float16)
    vals = None if novals else np.zeros((NCORES, NW, 128, Cmax), np.float32)

    # per-window slot layout (identical for every window): slot j maps to
    # (chunk crel, partition p), skipping reserved slots.  Which slots are
    # reserved depends only on gc0 % SEGC, and gc0 = w * Cmax.
    slot_cache = {}

    def slots_for(gc0):
        k = gc0 % SEGC
        if k not in slot_cache:
            s = np.arange(Cmax * 128)
            gcs = k + s // 128
            resv = ((gcs % SEGC) == SEGC - 1) & ((s % 128) == 127)
            slot_cache[k] = s[~resv]
        return slot_cache[k]

    for i, (c, w) in enumerate(cws):
        a, b = a_s[i], b_s[i]
        cnt = b - a
        if cnt == 0:
            continue
        gc0 = w * Cmax
        slots = slots_for(gc0)[:cnt]
        assert len(slots) == cnt, (c, w, cnt, Cmax)
        crel = slots // 128
        p = slots % 128
        src_i16[c, gc0 + crel, p] = (src_s[a:b] - 32768).astype(np.int16)
        tgt_i16[c, gc0 + crel, p] = (tgt_s[a:b] - 32768).astype(np.int16)
        tgtl[c, w, p, crel] = (tgt_s[a:b] - n0s[i]).astype(np.float16)
        if not novals:
            vals[c, w, p, crel] = val_s[a:b]

    # compact segment-packed idx arrays: [C, TSEG, 16, 64], value for gather
    # index j of segment s at [s, j % 16, j // 16]
    def pack(arr_i16):
        flat = np.zeros((NCORES, TSEG * SEG), np.int16)
        flat[:, :TC * 128] = arr_i16.reshape(NCORES, -1)
        return np.ascontiguousarray(
            flat.reshape(NCORES, TSEG, SEG // 16, 16).transpose(0, 1, 3, 2)
        ).reshape(NCORES * TSEG, 16, SEG // 16)

    idx_src = pack(src_i16)
    idx_tgt = pack(tgt_i16)

    # weights: wcat [128, 264] = [W (i->(h,o)) | ws | wt], fp16, replicated
    W = np.asarray(weight, np.float32)              # [4, 128, 64]
    aw = np.asarray(att_weight, np.float32)         # [4, 128]
    ws = np.stack([W[h] @ aw[h, :OUT_CH] for h in range(N_HEADS)], 1)
    wt = np.stack([W[h] @ aw[h, OUT_CH:] for h in range(N_HEADS)], 1)
    wcat1 = np.concatenate(
        [W.transpose(1, 0, 2).reshape(IN_CH, HO), ws, wt], 1).astype(np.float16)
    wcat = np.ascontiguousarray(np.broadcast_to(wcat1, (NCORES,) + wcat1.shape)
                                ).reshape(NCORES * IN_CH, HO + 8)

    # x, transposed + fp16 + sharded: core c gets columns [c*NPC, (c+1)*NPC)
    x_T = np.asarray(x_source, np.float16).T        # [128, 50000]
    x_sh = np.zeros((NCORES, IN_CH, NPCP), np.float16)
    for c in range(NCORES):
        x_sh[c, :, :NPC] = x_T[:, c * NPC:(c + 1) * NPC]
    x_sh = x_sh.reshape(NCORES * IN_CH, NPCP)

    tgtl = np.ascontiguousarray(tgtl.transpose(0, 2, 1, 3)
                                ).reshape(NCORES * 128, NW, Cmax)
    if not novals:
        vals = np.ascontiguousarray(vals.transpose(0, 2, 1, 3)
                                    ).reshape(NCORES * 128, NW, Cmax)
    return dict(Cmax=Cmax, TC=TC, TSEG=TSEG, novals=novals, x_sh=x_sh,
                wcat=wcat, idx_src=idx_src, idx_tgt=idx_tgt, tgtl=tgtl,
                vals=vals)


def _build(Cmax, TC, TSEG, novals):
    import concourse.bass as bass
    import concourse.tile as tile
    from concourse import bacc, mybir

    f32, f16, i16, i32 = (mybir.dt.float32, mybir.dt.float16,
                          mybir.dt.int16, mybir.dt.int32)
    Alu = mybir.AluOpType
    Act = mybir.ActivationFunctionType

    nc = bacc.Bacc("TRN2", target_bir_lowering=False, debug=False,
                   num_devices=NCORES, num_swdge_queues=1)
    x_sh = nc.dram_tensor("x_sh", [IN_CH, NPCP], f16, kind="ExternalInput")
    wcat = nc.dram_tensor("wcat", [IN_CH, HO + 8], f16, kind="ExternalInput")
    idx_src = nc.dram_tensor("idx_src", [TSEG, 16, SEG // 16], i16,
                             kind="ExternalInput")
    idx_tgt = nc.dram_tensor("idx_tgt", [TSEG, 16, SEG // 16], i16,
                             kind="ExternalInput")
    tgtl_in = nc.dram_tensor("tgtl", [128, NW, Cmax], f16, kind="ExternalInput")
    if not novals:
        vals_in = nc.dram_tensor("vals", [128, NW, Cmax], f32,
                                 kind="ExternalInput")
    out_d = nc.dram_tensor("out", [NPC, HO], f16, kind="ExternalOutput")
    xw_loc = nc.dram_tensor("xw_loc", [NPC, XROW], f16, kind="Internal")
    xw = nc.dram_tensor("xw", [N_NODES, XROW], f16, kind="Internal")

    with tile.TileContext(nc) as tc:
        # ---------------- phase A: local x_msg + AllGather ----------------
        with tc.tile_pool(name="a_w", bufs=1) as cpool, \
             tc.tile_pool(name="a_x", bufs=4) as xpool, \
             tc.tile_pool(name="a_ps", bufs=4, space="PSUM") as apsum, \
             tc.tile_pool(name="a_m", bufs=4) as mpool:
            wc = cpool.tile([128, HO + 8], f16)
            nc.sync.dma_start(wc[:], wcat[:])
            for i in range(NW):
                rows = min(128, NPC - i * 128)
                xt = xpool.tile([128, 128], f16)
                nc.sync.dma_start(xt[:], x_sh[:, i * 128:(i + 1) * 128])
                ps = apsum.tile([128, HO + 8], f32)
                nc.tensor.matmul(ps[:], xt[:], wc[:])
                m = mpool.tile([128, HO + 8], f16, tag="m")
                nc.vector.tensor_copy(m[0:rows, :], ps[0:rows, :])
                nc.sync.dma_start(xw_loc[i * 128:i * 128 + rows, 0:HO + 8],
                                  m[0:rows, :])

        tc.strict_bb_all_engine_barrier()
        nc.gpsimd.collective_compute(
            "AllGather", mybir.AluOpType.bypass,
            replica_groups=[list(range(NCORES))],
            ins=[xw_loc.ap().opt()], outs=[xw.ap().opt()])
        tc.strict_bb_all_engine_barrier()

        # ---------------- phase B ----------------
        with tc.tile_pool(name="b_c", bufs=1) as bconst, \
             tc.tile_pool(name="b_idx", bufs=16) as idxp, \
             tc.tile_pool(name="b_g", bufs=16) as gpool, \
             tc.tile_pool(name="b_t", bufs=16) as tpool, \
             tc.tile_pool(name="b_z", bufs=3) as zpool, \
             tc.tile_pool(name="b_oh", bufs=6) as ohpool, \
             tc.tile_pool(name="b_ps", bufs=2, space="PSUM") as bpsum, \
             tc.tile_pool(name="b_o", bufs=4) as opool:

            it32 = bconst.tile([128, 4 * 128], i32)
            nc.gpsimd.iota(it32[:], pattern=[[0, 4], [1, 128]],
                           channel_multiplier=0)
            iota4 = bconst.tile([128, 4, 128], f16)
            nc.vector.tensor_copy(iota4[:].rearrange("p a b -> p (a b)"), it32[:])
            bias_t = bconst.tile([128, 1], f32)
            nc.vector.memset(bias_t[:], EXP_BIAS)
            tl_all = bconst.tile([128, NW, Cmax], f16)
            nc.sync.dma_start(tl_all[:], tgtl_in[:])
            if not novals:
                vv_all = bconst.tile([128, NW, Cmax], f32)
                nc.sync.dma_start(vv_all[:], vals_in[:])

            tc.strict_bb_all_engine_barrier()

            seg_tiles = {}

            def get_seg(s):
                if s not in seg_tiles:
                    bs, bt = idx_src[s], idx_tgt[s]
                    si = idxp.tile([128, SEG // 16], i16, tag="si")
                    nc.sync.dma_start(
                        si[:], bass.AP(bs.tensor, bs.offset,
                                       [[0, 8]] + list(bs.ap)))
                    ti = idxp.tile([128, SEG // 16], i16, tag="ti")
                    nc.sync.dma_start(
                        ti[:], bass.AP(bt.tensor, bt.offset,
                                       [[0, 8]] + list(bt.ap)))
                    g = gpool.tile([128, SEGC, XROW], f16)
                    nc.gpsimd.dma_gather(g[:], xw[32768:, :], si[:], SEG, SEG,
                                         XROW, queue_num=0)
                    tg = tpool.tile([128, SEGC, TROW], f16)
                    nc.gpsimd.dma_gather(tg[:], xw[32768:, HO:HO + TROW], ti[:],
                                         SEG, SEG, TROW, elem_step=XROW,
                                         queue_num=0)
                    seg_tiles[s] = (g, tg)
                return seg_tiles[s]

            def bc(apv, n):
                return bass.AP(apv.tensor, apv.offset, list(apv.ap) + [[0, n]])

            for w in range(NW):
                rows = min(128, NPC - w * 128)
                tl = tl_all[:, w, :]

                gc0, gc1 = w * Cmax, (w + 1) * Cmax
                segs = sorted({gc // SEGC for gc in range(gc0, gc1)})

                # z = s + t (per segment range)
                z = zpool.tile([128, Cmax, N_HEADS], f32, tag="z")
                for s in segs:
                    lo, hi = max(s * SEGC, gc0), min(s * SEGC + SEGC, gc1)
                    g, tg = get_seg(s)
                    nc.vector.tensor_tensor(
                        z[:, lo - gc0:hi - gc0, :],
                        g[:, lo - s * SEGC:hi - s * SEGC, HO:HO + 4],
                        tg[:, lo - s * SEGC:hi - s * SEGC, 4:8], op=Alu.add)
                # lrelu
                zz = zpool.tile([128, Cmax, N_HEADS], f32, tag="zz")
                nc.vector.scalar_tensor_tensor(
                    zz[:].rearrange("p c h -> p (c h)"),
                    z[:].rearrange("p c h -> p (c h)"), 0.01,
                    z[:].rearrange("p c h -> p (c h)"),
                    op0=Alu.mult, op1=Alu.max)
                if not novals:
                    vv = vv_all[:, w, :]
                    nc.vector.tensor_tensor(zz[:], zz[:], bc(vv, N_HEADS),
                                            op=Alu.mult)
                # p = exp(zz - 4)
                p = zpool.tile([128, Cmax, N_HEADS], f16, tag="p")
                nc.scalar.activation(p[:], zz[:], Act.Exp, bias=bias_t[:])

                # rhs in-place: g.msg *= p ; g.s <- p
                for s in segs:
                    lo, hi = max(s * SEGC, gc0), min(s * SEGC + SEGC, gc1)
                    g, _ = get_seg(s)
                    gm = g[:, lo - s * SEGC:hi - s * SEGC, 0:HO].rearrange(
                        "p c (h o) -> p c h o", o=OUT_CH)
                    nc.vector.tensor_tensor(
                        gm, gm, bc(p[:, lo - gc0:hi - gc0, :], OUT_CH),
                        op=Alu.mult)
                    nc.vector.tensor_copy(
                        g[:, lo - s * SEGC:hi - s * SEGC, HO:HO + 4],
                        p[:, lo - gc0:hi - gc0, :])

                ps = bpsum.tile([128, HO + 4], f32)
                for cb in range(0, Cmax, 4):
                    nb = min(4, Cmax - cb)
                    oh = ohpool.tile([128, 4, 128], f16)
                    nc.vector.tensor_tensor(
                        oh[:, 0:nb, :], iota4[:, 0:nb, :],
                        bc(tl[:, cb:cb + nb], 128), op=Alu.is_equal)
                    for j in range(nb):
                        c = cb + j
                        gc = gc0 + c
                        g, _ = get_seg(gc // SEGC)
                        nc.tensor.matmul(
                            ps[:], oh[:, j, :],
                            g[:, gc % SEGC, 0:HO + 4],
                            start=(c == 0), stop=(c == Cmax - 1))

                d = opool.tile([128, 4], f32, tag="d")
                nc.vector.tensor_scalar_max(d[:], ps[:, HO:HO + 4], 1e-30)
                r = opool.tile([128, 4], f32, tag="r")
                nc.vector.reciprocal(r[:], d[:])
                o = opool.tile([128, HO], f16, tag="o")
                nc.vector.tensor_tensor(
                    o[:].rearrange("p (h q) -> p h q", q=OUT_CH),
                    ps[:, 0:HO].rearrange("p (h q) -> p h q", q=OUT_CH),
                    bc(r[:], OUT_CH), op=Alu.mult)
                nc.sync.dma_start(out_d[w * 128:w * 128 + rows, :], o[0:rows, :])

    nc.finalize()
    return nc


_CACHE = {}


def _get_runner(Cmax, TC, TSEG, novals):
    key = (Cmax, TC, TSEG, novals)
    if key in _CACHE:
        return _CACHE[key]
    import jax
    from concourse import mybir
    from concourse.bass2jax import (_bass_exec_p, install_neuronx_cc_hook,
                                    partition_id_tensor)
    from jax.sharding import Mesh, PartitionSpec
    from jax.experimental.shard_map import shard_map

    nc = _build(Cmax, TC, TSEG, novals)
    install_neuronx_cc_hook()
    partition_name = (nc.partition_id_tensor.name
                      if nc.partition_id_tensor else None)
    in_names, out_names, out_avals = [], [], []
    for alloc in nc.m.functions[0].allocations:
        if not isinstance(alloc, mybir.MemoryLocationSet):
            continue
        name = alloc.memorylocations[0].name
        if alloc.kind == "ExternalInput":
            if name != partition_name:
                in_names.append(name)
        elif alloc.kind == "ExternalOutput":
            out_names.append(name)
            out_avals.append(jax.core.ShapedArray(
                tuple(alloc.tensor_shape), mybir.dt.np(alloc.dtype)))
    all_names = list(in_names) + ([partition_name] if partition_name else [])

    def _body(*args):
        operands = list(args)
        if partition_name is not None:
            operands.append(partition_id_tensor())
        return tuple(_bass_exec_p.bind(
            *operands, out_avals=tuple(out_avals), in_names=tuple(all_names),
            out_names=tuple(out_names), lowering_input_output_aliases=(),
            sim_require_finite=True, sim_require_nnan=True, nc=nc))

    devices = jax.devices()[:NCORES]
    mesh = Mesh(np.asarray(devices), ("core",))
    sharded = jax.jit(shard_map(
        _body, mesh=mesh, in_specs=(PartitionSpec("core"),) * len(in_names),
        out_specs=(PartitionSpec("core"),) * len(out_names), check_rep=False))
    _CACHE[key] = (sharded, in_names)
    return _CACHE[key]


def kernel(x_source, edge_tgt, edge_src, edge_vals, weight, att_weight):
    import time
    prep = _host_prep(np.asarray(x_source), np.asarray(edge_tgt),
                      np.asarray(edge_src), np.asarray(edge_vals),
                      np.asarray(weight), np.asarray(att_weight))
    sharded, in_names = _get_runner(prep["Cmax"], prep["TC"], prep["TSEG"],
                                    prep["novals"])
    t0 = time.time()
    outs = sharded(*[prep[n] for n in in_names])
    out16 = np.asarray(outs[0])
    kernel.last_run_wall_s = time.time() - t0
    return out16.astype(np.float32)
